# revision 40
# baseline (speedup 1.0000x reference)
"""Causal self-attention (b=2, t=2048, d=1024, h=16) on 8 trn2 NeuronCores.

Sharding: core c handles batch c//4 and the 4 heads 4*(c%4)..4*(c%4)+3
(data parallel over batch x tensor parallel over heads). Each core
computes x @ w_qkv for its head-slice, causal attention for its heads,
and a partial out-projection  y_heads @ w_out[head_rows]; the host sums
the 4 partial outputs per batch (the tensor-parallel all-reduce).

Layout/perf notes:
  x is transposed on the HOST (f32) so the kernel does plain contiguous
  DMAs into f32r tiles (no DMA-transpose, no hi/lo bf16 split, no DVE
  merge). Weights are host-swizzled to [128, chunks, n].
  Input DMAs are spread across the Sync/Scalar/GpSimd queues so issue
  (~1.3us each) does not serialize the head of the kernel.
  qT, kT [dh, t] f32r per head-pair (128 partitions = 2 heads x 64).
  S^T is computed per (i-block 512, j-chunk 128) into a 2-bank PSUM tile
  holding BOTH heads of the pair; one ACT instr exps both heads into a
  bf16 P tile (bf16 moving operand keeps 1 cyc/row even for the 128-wide
  diagonal chunks). V is bf16 with a fused ones column so the PV matmul
  emits y_unnorm and the softmax denominator together; scores are O(5)
  so exp needs no max-subtraction. Softmax renorm: rec = exp(-ln D) on
  ACT (activation tables reordered so Exp and Ln share one table set),
  broadcast across partitions on the idle GpSimd engine, multiplied in
  on DVE. Output collects in one bf16 SBUF tile, stored in 4 big DMAs,
  summed in f32 on the host.
"""

import numpy as np
import ml_dtypes

import concourse.bacc as bacc
import concourse.hw_specs as hw_specs
import concourse.mybir as mybir
import concourse.tile as tile
from concourse.bass_utils import run_bass_kernel_spmd

F32 = mybir.dt.float32
F32R = mybir.dt.float32r
BF16 = mybir.dt.bfloat16
AF = mybir.ActivationFunctionType

T = 2048            # sequence length
D = 1024            # model dim
DH = 64             # head dim
HPC = 4             # heads per core
NCORES = 8
NTT = T // 128      # 16 t-tiles of 128
NDC = D // 128      # 8 d-chunks of 128
NIB = T // 512      # 4 i-blocks of 512
JPB = 512 // 128    # j-chunks per i-block
VW = DH + 2         # v row stride: 64 v + 1 ones + 1 pad (4B alignment)

_TABLES_PATCHED = False


def _patch_act_tables():
    """Prefer natural_log_exp_and_others so Exp and Ln activations share
    one table set (otherwise the per-renorm Ln thrashes ~2.7us reloads)."""
    global _TABLES_PATCHED
    if _TABLES_PATCHED:
        return
    _TABLES_PATCHED = True
    orig = hw_specs.get_activation_tables

    def patched(arch):
        # act_func_set_id is positional (index into act_info.json), so the
        # dict order/size must be preserved. Steer the chooser by removing
        # Exp/Ln from every OTHER set, so both resolve to the shared set.
        tabs = dict(orig(arch))
        pref = "natural_log_exp_and_others"
        if pref in tabs:
            drop = {AF.Exp, AF.Ln}
            tabs = {k: (v if k == pref else set(v) - drop)
                    for k, v in tabs.items()}
        return tabs

    hw_specs.get_activation_tables = patched
    bacc.get_activation_tables = patched


def _build():
    _patch_act_tables()
    nc = bacc.Bacc("TRN2", target_bir_lowering=False, debug=False)

    XT = nc.dram_tensor("XT", [128, NDC, T], BF16, kind="ExternalInput")
    WQ = nc.dram_tensor("WQ", [128, NDC, 256], BF16, kind="ExternalInput")
    WK = nc.dram_tensor("WK", [128, NDC, 256], BF16, kind="ExternalInput")
    WV = nc.dram_tensor("WV", [128, NDC, 256], BF16, kind="ExternalInput")
    WO = nc.dram_tensor("WO", [128, 2, D], BF16, kind="ExternalInput")
    TRI = nc.dram_tensor("TRI", [128, 128], BF16, kind="ExternalInput")
    OUT = nc.dram_tensor("OUT", [128, NTT, D], BF16, kind="ExternalOutput")

    with tile.TileContext(nc) as tc:
        with tc.tile_pool(name="persist", bufs=1) as pp:
            xt = pp.tile([128, NDC, T], BF16, tag="xt")
            wq_sb = pp.tile([128, NDC, 256], BF16, tag="wq")
            wk_sb = pp.tile([128, NDC, 256], BF16, tag="wk")
            wv_sb = pp.tile([128, NDC, 256], BF16, tag="wv")
            wo_sb = pp.tile([128, 2, D], BF16, tag="wo")
            qt = [pp.tile([128, T], F32R, tag=f"qt{p}", name=f"qt{p}")
                  for p in range(2)]
            kt = [pp.tile([128, T], F32R, tag=f"kt{p}", name=f"kt{p}")
                  for p in range(2)]
            vones = pp.tile([128, NTT, HPC, VW], BF16, tag="vones")
            ones1 = pp.tile([1, 64], F32R, tag="ones1")
            ypair = [pp.tile([128, T], BF16, tag=f"yp{p}", name=f"yp{p}")
                     for p in range(2)]
            tri = pp.tile([128, 128], BF16, tag="tri")

            # input DMAs spread across three issue queues so descriptor
            # generation (~0.6-1.3us per dma_start) runs in parallel;
            # within each queue, earliest-needed first.
            # x chunks on one queue (so they ARRIVE in chain order),
            # weights on the other; constant tiles are built on-chip.
            for tb in range(NIB):
                ts_ = slice(tb * 512, (tb + 1) * 512)
                nc.sync.dma_start(xt[:, :, ts_], XT[:, :, ts_])
            nc.scalar.dma_start(wv_sb[:], WV[:])
            nc.scalar.dma_start(wq_sb[:], WQ[:])
            nc.scalar.dma_start(wk_sb[:], WK[:])
            nc.scalar.dma_start(wo_sb[:], WO[:])
            nc.scalar.dma_start(tri[:], TRI[:])
            # ones col (v evac overwrites the data region); int bit patterns
            nc.gpsimd.memset(vones[:].bitcast(mybir.dt.uint16), 0x3F80)
            nc.gpsimd.memset(ones1[:].bitcast(mybir.dt.uint32), 0x3F800000)

            # -------- PE clock warmup --------
            # ~60 back-to-back tiny matmuls on a zeroed junk tile keep the
            # PE busy >3.4us with no data deps, so the HAM un-throttles
            # the clock (1.2 -> 2.4 GHz) before the real matmuls start.
            with tc.tile_pool(name="warm", bufs=1) as pw, \
                 tc.tile_pool(name="pswarm", bufs=1, space="PSUM") as psw:
                junk = pw.tile([128, 64], BF16, tag="junk")
                nc.vector.memset(junk[:], 0.0)
                jp = psw.tile([64, 64], F32, tag="junkp")
                for i in range(60):
                    nc.tensor.matmul(jp[:], junk[:, 0:64], junk[:, 0:64],
                                     start=True, stop=True)
                # read the result so DCE keeps the chain
                nc.vector.tensor_copy(junk[0:64, 0:1], jp[:, 0:1])

            # -------- shared phase-B helpers (pools opened outer) --------
            with tc.tile_pool(name="phBpt", bufs=4) as pbpt, \
                 tc.tile_pool(name="phBn", bufs=1) as pbn, \
                 tc.tile_pool(name="phC", bufs=2) as pc_:
                def emit_block(pi, ib, yab, stab_alloc, hook=None):
                    """S/exp/mask/PV for block (pi, ib); PV pipelined one
                    chunk behind S so the in-order PE stays fed."""
                    jlast = JPB * ib + JPB - 1
                    prev_pv = None
                    for jc in range(jlast + 1):
                        off = 128 * (jc - JPB * ib) if jc >= JPB * ib else 0
                        stab = stab_alloc()
                        ptab = pbpt.tile([128, 2, 512], BF16, tag="ptab")
                        js = slice(jc * 128, (jc + 1) * 128)
                        isl = slice(ib * 512 + off, (ib + 1) * 512)
                        nc.tensor.matmul(
                            stab[:, 0, off:512], kt[pi][0:64, js],
                            qt[pi][0:64, isl], start=True, stop=True)
                        nc.tensor.matmul(
                            stab[:, 1, off:512], kt[pi][64:128, js],
                            qt[pi][64:128, isl], start=True, stop=True,
                            tile_position=(64, 0))
                        nc.scalar.activation(
                            ptab[:, :, off:512], stab[:, :, off:512],
                            AF.Exp, scale=0.125)
                        if jc >= JPB * ib:  # diagonal chunk: mask triangle
                            nc.vector.tensor_mul(
                                ptab[:, 0, off:off + 128],
                                ptab[:, 0, off:off + 128], tri[:])
                            nc.vector.tensor_mul(
                                ptab[:, 1, off:off + 128],
                                ptab[:, 1, off:off + 128], tri[:])
                        if prev_pv is not None:
                            prev_pv()
                        if hook is not None:
                            hook(jc)

                        def mk_pv(yab, jc, off, pi, ptab):
                            def pv():
                                for h in range(2):
                                    nc.tensor.matmul(
                                        yab[0:65, h, off:512],
                                        vones[:, jc, 2 * pi + h, 0:DH + 1],
                                        ptab[:, h, off:512],
                                        start=(jc == 0), stop=(jc == jlast))
                            return pv
                        prev_pv = mk_pv(yab, jc, off, pi, ptab)
                    prev_pv()

                def make_renorm(pi, ib, yab):
                    def renorm(bc_pool, bc_tag):
                        ibs = slice(ib * 512, (ib + 1) * 512)
                        lnd = pbn.tile([1, 2, 512], F32R, tag="lnd",
                                       name="lnd")
                        nc.scalar.activation(lnd[:], yab[64:65, :, :], AF.Ln)
                        for h in range(2):
                            bc = bc_pool.tile([64, 512], F32, tag=bc_tag,
                                              name=f"bc{h}")
                            bcs = pbn.tile([64, 512], F32R, tag=f"bcs{h}",
                                           name=f"bcs{h}")
                            nc.tensor.matmul(
                                bc[:], ones1[:], lnd[0:1, h, :],
                                start=True, stop=True)
                            nc.scalar.activation(
                                bcs[:], bc[:], AF.Exp, scale=-1.0)
                            nc.vector.tensor_mul(
                                ypair[pi][64 * h:64 * h + 64, ibs],
                                yab[0:64, h, :], bcs[:])
                    return renorm

                # -------- phase A head, t-blocked --------
                # Each 512-wide t-block of x arrives as one DMA and
                # immediately completes its q0/k0 chains; attention block
                # (0,0) is emitted right after t-block 0 in a small
                # dedicated PSUM scope, so ACT starts ~15us earlier.
                pending00 = None
                with tc.tile_pool(name="psqk", bufs=2, space="PSUM") as psqk, \
                     tc.tile_pool(name="psv", bufs=2, space="PSUM") as psv:
                    def tb_chains(tb):
                        ts_ = slice(tb * 512, (tb + 1) * 512)
                        qpq = psqk.tile([128, 512], F32, tag="qkp",
                                        name="qpq")
                        qpk = psqk.tile([128, 512], F32, tag="qkp",
                                        name="qpk")
                        for dc in range(NDC):
                            nc.tensor.matmul(
                                qpq[:], wq_sb[:, dc, 0:128],
                                xt[:, dc, ts_],
                                start=(dc == 0), stop=(dc == NDC - 1))
                            nc.tensor.matmul(
                                qpk[:], wk_sb[:, dc, 0:128],
                                xt[:, dc, ts_],
                                start=(dc == 0), stop=(dc == NDC - 1))
                        nc.vector.tensor_copy(qt[0][:, ts_], qpq[:])
                        nc.vector.tensor_copy(kt[0][:, ts_], qpk[:])

                    def v_proj_a(ti):
                        vpt = psv.tile([128, 256], F32, tag="vp", name="vpt")
                        for dc in range(NDC):
                            nc.tensor.matmul(
                                vpt[:], xt[:, dc, ti * 128:(ti + 1) * 128],
                                wv_sb[:, dc, :],
                                start=(dc == 0), stop=(dc == NDC - 1))
                        nc.vector.tensor_copy(
                            vones[:, ti, :, 0:DH],
                            vpt[:].rearrange("p (h d) -> p h d", h=HPC))

                    tb_chains(0)
                    for ti in range(4):
                        v_proj_a(ti)
                    # early attention block (0,0): 4 banks alongside A's 4
                    with tc.tile_pool(name="psB0", bufs=1,
                                      space="PSUM") as psb0, \
                         tc.tile_pool(name="psY0", bufs=1,
                                      space="PSUM") as psy0:
                        yab00 = psy0.tile([128, 2, 512], F32, tag="yab0")
                        emit_block(
                            0, 0, yab00,
                            lambda: psb0.tile([128, 2, 512], F32,
                                              tag="stab0", name="st0"))
                        pending00 = make_renorm(0, 0, yab00)
                        tb_chains(1)
                        tb_chains(2)
                        tb_chains(3)
                        for ti in range(4, 8):
                            v_proj_a(ti)

            # ------------- phase B main: blocks (0,1)..(1,3) -------
                with tc.tile_pool(name="psBst", bufs=2, space="PSUM") as psbst, \
                     tc.tile_pool(name="psBy", bufs=2, space="PSUM") as psby:
                    def v_proj_b(ti):
                        vp = psbst.tile([128, 256], F32, tag="stab",
                                        name="vpb")
                        for dc in range(NDC):
                            nc.tensor.matmul(
                                vp[:], xt[:, dc, ti * 128:(ti + 1) * 128],
                                wv_sb[:, dc, :],
                                start=(dc == 0), stop=(dc == NDC - 1))
                        nc.vector.tensor_copy(
                            vones[:, ti, :, 0:DH],
                            vp[:].rearrange("p (h d) -> p h d", h=HPC))

                    def qk_chain(w_sb, dst, pi, ib):
                        qp = psby.tile([128, 512], F32, tag="yab",
                                       name="qpc")
                        for dc in range(NDC):
                            nc.tensor.matmul(
                                qp[:], w_sb[:, dc, pi * 128:(pi + 1) * 128],
                                xt[:, dc, ib * 512:(ib + 1) * 512],
                                start=(dc == 0), stop=(dc == NDC - 1))
                        nc.vector.tensor_copy(
                            dst[pi][:, ib * 512:(ib + 1) * 512], qp[:])

                    ostg = {}

                    def c_pair(g, tq):
                        # out-projection for t-tile 4g+tq (both halves)
                        if tq == 0:
                            ostg[g] = pc_.tile([128, 4, D], BF16, tag="ostg",
                                               name=f"ostg{g}")
                        ti = 4 * g + tq
                        for eh in range(2):
                            op = psby.tile([128, 512], F32, tag="yab",
                                           name=f"op{eh}")
                            for pi in range(2):
                                nc.tensor.matmul(
                                    op[:],
                                    ypair[pi][:, ti * 128:(ti + 1) * 128],
                                    wo_sb[:, pi, eh * 512:(eh + 1) * 512],
                                    start=(pi == 0), stop=(pi == 1))
                            nc.vector.tensor_copy(
                                ostg[g][:, tq, eh * 512:(eh + 1) * 512],
                                op[:])
                        if g == NIB - 1:  # tail group: store per-ti
                            nc.sync.dma_start(
                                OUT[:, ti:ti + 1, :],
                                ostg[g][:, tq:tq + 1, :])
                        elif tq == 3:
                            nc.sync.dma_start(
                                OUT[:, 4 * g:4 * g + 4, :], ostg[g][:])

                    FILLERS = {
                        (0, 1): [lambda: [v_proj_b(t) for t in (8, 9)],
                                 lambda: [v_proj_b(t) for t in (10, 11)]],
                        (0, 2): [lambda: [v_proj_b(t) for t in (12, 13)],
                                 lambda: [v_proj_b(t) for t in (14, 15)],
                                 lambda: qk_chain(wq_sb, qt, 1, 0),
                                 lambda: qk_chain(wk_sb, kt, 1, 0)],
                        (0, 3): [lambda: qk_chain(wq_sb, qt, 1, 1),
                                 lambda: qk_chain(wk_sb, kt, 1, 1)],
                        (1, 0): [lambda: qk_chain(wq_sb, qt, 1, 2),
                                 lambda: qk_chain(wk_sb, kt, 1, 2)],
                        (1, 1): [lambda: qk_chain(wq_sb, qt, 1, 3),
                                 lambda: qk_chain(wk_sb, kt, 1, 3)],
                    }
                    state = {"pending": pending00, "cgrp": None,
                             "cslots": ()}

                    def hook(pi, ib):
                        def h(jc):
                            if jc == 1 and state["pending"] is not None:
                                state["pending"](psbst, "stab")
                                state["pending"] = None
                                if pi == 1 and ib >= 1:
                                    state["cgrp"] = ib - 1
                                    state["cslots"] = (
                                        (3, 5, 6, 7) if JPB * ib + JPB < 10
                                        else (3, 5, 7, 9))
                            if (state["cgrp"] is not None
                                    and jc in state["cslots"]):
                                c_pair(state["cgrp"],
                                       state["cslots"].index(jc))
                        return h

                    for pi in range(2):
                        for ib in range(NIB):
                            if pi == 0 and ib == 0:
                                continue  # emitted early in phase A
                            yab = psby.tile([128, 2, 512], F32, tag="yab",
                                            name="yab")
                            emit_block(
                                pi, ib, yab,
                                lambda: psbst.tile([128, 2, 512], F32,
                                                   tag="stab", name="stab"),
                                hook(pi, ib))
                            state["pending"] = make_renorm(pi, ib, yab)
                            for fn in FILLERS.get((pi, ib), ()):
                                fn()
                    state["pending"](psbst, "stab")
                    for tq in range(4):
                        c_pair(NIB - 1, tq)

    nc.compile()
    return nc


_NC = None


def build_in_maps(x, w_qkv, w_out):
    x = np.asarray(x, np.float32)
    w_qkv = np.asarray(w_qkv, np.float32)
    w_out = np.asarray(w_out, np.float32)

    tri = np.triu(np.ones((128, 128), np.float32)).astype(
        ml_dtypes.bfloat16)                                # tri[j,i]=1 iff j<=i

    # [d, n] -> [128, d//128, n] with partition p: d = chunk*128 + p
    def dswz(w, dt=np.float32):
        return np.ascontiguousarray(
            w.reshape(NDC, 128, -1).transpose(1, 0, 2)).astype(dt)

    in_maps = []
    for c in range(NCORES):
        b, g = divmod(c, 4)
        cs = slice(g * 256, (g + 1) * 256)
        in_maps.append({
            "XT": dswz(np.ascontiguousarray(x[b].T), ml_dtypes.bfloat16),
            "WQ": dswz(np.ascontiguousarray(w_qkv[:, 0:1024][:, cs]),
                       ml_dtypes.bfloat16),
            "WK": dswz(np.ascontiguousarray(w_qkv[:, 1024:2048][:, cs]),
                       ml_dtypes.bfloat16),
            "WV": dswz(np.ascontiguousarray(w_qkv[:, 2048:3072][:, cs]),
                       ml_dtypes.bfloat16),
            "WO": np.ascontiguousarray(
                w_out[g * 256:(g + 1) * 256, :].reshape(2, 128, D)
                .transpose(1, 0, 2)).astype(ml_dtypes.bfloat16),
            "TRI": tri,
        })
    return in_maps


def kernel(x, w_qkv, w_out):
    global _NC
    if _NC is None:
        _NC = _build()

    in_maps = build_in_maps(x, w_qkv, w_out)
    res = run_bass_kernel_spmd(_NC, in_maps, core_ids=list(range(NCORES)))
    # OUT is [128, NTT, D] with row t = ti*128 + p -> unswizzle to [T, D]
    outs = [res.results[c]["OUT"].astype(np.float32)
            .transpose(1, 0, 2).reshape(T, D) for c in range(NCORES)]
    y = np.stack([outs[0] + outs[1] + outs[2] + outs[3],
                  outs[4] + outs[5] + outs[6] + outs[7]], axis=0)
    return y.astype(np.float32)


# revision 44
# speedup vs baseline: 1.0138x; 1.0138x over previous
"""Causal self-attention (b=2, t=2048, d=1024, h=16) on 8 trn2 NeuronCores.

Sharding: core c handles batch c//4 and the 4 heads 4*(c%4)..4*(c%4)+3
(data parallel over batch x tensor parallel over heads). Each core
computes x @ w_qkv for its head-slice, causal attention for its heads,
and a partial out-projection  y_heads @ w_out[head_rows]; the host sums
the 4 partial outputs per batch (the tensor-parallel all-reduce).

Layout/perf notes:
  x is transposed on the HOST (f32) so the kernel does plain contiguous
  DMAs into f32r tiles (no DMA-transpose, no hi/lo bf16 split, no DVE
  merge). Weights are host-swizzled to [128, chunks, n].
  Input DMAs are spread across the Sync/Scalar/GpSimd queues so issue
  (~1.3us each) does not serialize the head of the kernel.
  qT, kT [dh, t] f32r per head-pair (128 partitions = 2 heads x 64).
  S^T is computed per (i-block 512, j-chunk 128) into a 2-bank PSUM tile
  holding BOTH heads of the pair; one ACT instr exps both heads into a
  bf16 P tile (bf16 moving operand keeps 1 cyc/row even for the 128-wide
  diagonal chunks). V is bf16 with a fused ones column so the PV matmul
  emits y_unnorm and the softmax denominator together; scores are O(5)
  so exp needs no max-subtraction. Softmax renorm: rec = exp(-ln D) on
  ACT (activation tables reordered so Exp and Ln share one table set),
  broadcast across partitions on the idle GpSimd engine, multiplied in
  on DVE. Output collects in one bf16 SBUF tile, stored in 4 big DMAs,
  summed in f32 on the host.
"""

import numpy as np
import ml_dtypes

import concourse.bacc as bacc
import concourse.hw_specs as hw_specs
import concourse.mybir as mybir
import concourse.tile as tile
from concourse.bass_utils import run_bass_kernel_spmd

F32 = mybir.dt.float32
F32R = mybir.dt.float32r
BF16 = mybir.dt.bfloat16
AF = mybir.ActivationFunctionType

T = 2048            # sequence length
D = 1024            # model dim
DH = 64             # head dim
HPC = 4             # heads per core
NCORES = 8
NTT = T // 128      # 16 t-tiles of 128
NDC = D // 128      # 8 d-chunks of 128
NIB = T // 512      # 4 i-blocks of 512
JPB = 512 // 128    # j-chunks per i-block
VW = DH + 2         # v row stride: 64 v + 1 ones + 1 pad (4B alignment)

_TABLES_PATCHED = False


def _patch_act_tables():
    """Prefer natural_log_exp_and_others so Exp and Ln activations share
    one table set (otherwise the per-renorm Ln thrashes ~2.7us reloads)."""
    global _TABLES_PATCHED
    if _TABLES_PATCHED:
        return
    _TABLES_PATCHED = True
    orig = hw_specs.get_activation_tables

    def patched(arch):
        # act_func_set_id is positional (index into act_info.json), so the
        # dict order/size must be preserved. Steer the chooser by removing
        # Exp/Ln from every OTHER set, so both resolve to the shared set.
        tabs = dict(orig(arch))
        pref = "natural_log_exp_and_others"
        if pref in tabs:
            drop = {AF.Exp, AF.Ln}
            tabs = {k: (v if k == pref else set(v) - drop)
                    for k, v in tabs.items()}
        return tabs

    hw_specs.get_activation_tables = patched
    bacc.get_activation_tables = patched


def _build():
    _patch_act_tables()
    nc = bacc.Bacc("TRN2", target_bir_lowering=False, debug=False)

    XT = nc.dram_tensor("XT", [128, NDC, T], BF16, kind="ExternalInput")
    WQ = nc.dram_tensor("WQ", [128, NDC, 256], BF16, kind="ExternalInput")
    WK = nc.dram_tensor("WK", [128, NDC, 256], BF16, kind="ExternalInput")
    WV = nc.dram_tensor("WV", [128, NDC, 256], BF16, kind="ExternalInput")
    WO = nc.dram_tensor("WO", [128, 2, D], BF16, kind="ExternalInput")
    TRI = nc.dram_tensor("TRI", [128, 128], BF16, kind="ExternalInput")
    OUT = nc.dram_tensor("OUT", [128, NTT, D], BF16, kind="ExternalOutput")

    with tile.TileContext(nc) as tc:
        with tc.tile_pool(name="persist", bufs=1) as pp:
            xt = pp.tile([128, NDC, T], BF16, tag="xt")
            wq_sb = pp.tile([128, NDC, 256], BF16, tag="wq")
            wk_sb = pp.tile([128, NDC, 256], BF16, tag="wk")
            wv_sb = pp.tile([128, NDC, 256], BF16, tag="wv")
            wo_sb = pp.tile([128, 2, D], BF16, tag="wo")
            qt = [pp.tile([128, T], F32R, tag=f"qt{p}", name=f"qt{p}")
                  for p in range(2)]
            kt = [pp.tile([128, T], F32R, tag=f"kt{p}", name=f"kt{p}")
                  for p in range(2)]
            vones = pp.tile([128, NTT, HPC, VW], BF16, tag="vones")
            ones1 = pp.tile([1, 64], F32R, tag="ones1")
            ypair = [pp.tile([128, T], BF16, tag=f"yp{p}", name=f"yp{p}")
                     for p in range(2)]
            tri = pp.tile([128, 128], BF16, tag="tri")

            # input DMAs spread across three issue queues so descriptor
            # generation (~0.6-1.3us per dma_start) runs in parallel;
            # within each queue, earliest-needed first.
            # x chunks on one queue (so they ARRIVE in chain order),
            # weights on the other; constant tiles are built on-chip.
            for tb in range(NIB):
                ts_ = slice(tb * 512, (tb + 1) * 512)
                nc.sync.dma_start(xt[:, :, ts_], XT[:, :, ts_])
            nc.scalar.dma_start(wv_sb[:], WV[:])
            nc.scalar.dma_start(wq_sb[:], WQ[:])
            nc.scalar.dma_start(wk_sb[:], WK[:])
            nc.scalar.dma_start(wo_sb[:], WO[:])
            nc.scalar.dma_start(tri[:], TRI[:])
            # ones col (v evac overwrites the data region); int bit patterns
            nc.gpsimd.memset(vones[:].bitcast(mybir.dt.uint16), 0x3F80)
            nc.gpsimd.memset(ones1[:].bitcast(mybir.dt.uint32), 0x3F800000)

            # -------- PE clock warmup --------
            # ~60 back-to-back tiny matmuls on a zeroed junk tile keep the
            # PE busy >3.4us with no data deps, so the HAM un-throttles
            # the clock (1.2 -> 2.4 GHz) before the real matmuls start.
            with tc.tile_pool(name="warm", bufs=1) as pw, \
                 tc.tile_pool(name="pswarm", bufs=1, space="PSUM") as psw:
                junk = pw.tile([128, 64], BF16, tag="junk")
                nc.vector.memset(junk[:], 0.0)
                jp = psw.tile([64, 64], F32, tag="junkp")
                for i in range(60):
                    nc.tensor.matmul(jp[:], junk[:, 0:64], junk[:, 0:64],
                                     start=True, stop=True)
                # read the result so DCE keeps the chain
                nc.vector.tensor_copy(junk[0:64, 0:1], jp[:, 0:1])

            # -------- shared phase-B helpers (pools opened outer) --------
            with tc.tile_pool(name="phBpt", bufs=4) as pbpt, \
                 tc.tile_pool(name="phBn", bufs=1) as pbn, \
                 tc.tile_pool(name="phC", bufs=2) as pc_:
                def emit_block(pi, ib, yab, stab_alloc, hook=None):
                    """S/exp/mask/PV for block (pi, ib); PV pipelined one
                    chunk behind S so the in-order PE stays fed."""
                    jlast = JPB * ib + JPB - 1
                    prev_pv = None
                    for jc in range(jlast + 1):
                        off = 128 * (jc - JPB * ib) if jc >= JPB * ib else 0
                        stab = stab_alloc()
                        ptab = pbpt.tile([128, 2, 512], BF16, tag="ptab")
                        js = slice(jc * 128, (jc + 1) * 128)
                        isl = slice(ib * 512 + off, (ib + 1) * 512)
                        nc.tensor.matmul(
                            stab[:, 0, off:512], kt[pi][0:64, js],
                            qt[pi][0:64, isl], start=True, stop=True)
                        nc.tensor.matmul(
                            stab[:, 1, off:512], kt[pi][64:128, js],
                            qt[pi][64:128, isl], start=True, stop=True,
                            tile_position=(64, 0))
                        nc.scalar.activation(
                            ptab[:, :, off:512], stab[:, :, off:512],
                            AF.Exp, scale=0.125)
                        if jc >= JPB * ib:  # diagonal chunk: mask triangle
                            nc.vector.tensor_mul(
                                ptab[:, 0, off:off + 128],
                                ptab[:, 0, off:off + 128], tri[:])
                            nc.vector.tensor_mul(
                                ptab[:, 1, off:off + 128],
                                ptab[:, 1, off:off + 128], tri[:])
                        if prev_pv is not None:
                            prev_pv()
                        if hook is not None:
                            hook(jc)

                        def mk_pv(yab, jc, off, pi, ptab):
                            def pv():
                                for h in range(2):
                                    nc.tensor.matmul(
                                        yab[0:65, h, off:512],
                                        vones[:, jc, 2 * pi + h, 0:DH + 1],
                                        ptab[:, h, off:512],
                                        start=(jc == 0), stop=(jc == jlast))
                            return pv
                        prev_pv = mk_pv(yab, jc, off, pi, ptab)
                    prev_pv()

                def make_renorm(pi, ib, yab):
                    def renorm(bc_pool, bc_tag):
                        ibs = slice(ib * 512, (ib + 1) * 512)
                        lnd = pbn.tile([1, 2, 512], F32R, tag="lnd",
                                       name="lnd")
                        nc.scalar.activation(lnd[:], yab[64:65, :, :], AF.Ln)
                        for h in range(2):
                            bc = bc_pool.tile([64, 512], F32, tag=bc_tag,
                                              name=f"bc{h}")
                            bcs = pbn.tile([64, 512], F32R, tag=f"bcs{h}",
                                           name=f"bcs{h}")
                            nc.tensor.matmul(
                                bc[:], ones1[:], lnd[0:1, h, :],
                                start=True, stop=True)
                            nc.scalar.activation(
                                bcs[:], bc[:], AF.Exp, scale=-1.0)
                            nc.vector.tensor_mul(
                                ypair[pi][64 * h:64 * h + 64, ibs],
                                yab[0:64, h, :], bcs[:])
                    return renorm

                # -------- phase A head, t-blocked --------
                # Each 512-wide t-block of x arrives as one DMA and
                # immediately completes its q0/k0 chains; attention block
                # (0,0) is emitted right after t-block 0 in a small
                # dedicated PSUM scope, so ACT starts ~15us earlier.
                with tc.tile_pool(name="psqk", bufs=2, space="PSUM") as psqk, \
                     tc.tile_pool(name="psv", bufs=2, space="PSUM") as psv:
                    def tb_chains(tb):
                        ts_ = slice(tb * 512, (tb + 1) * 512)
                        qpq = psqk.tile([128, 512], F32, tag="qkp",
                                        name="qpq")
                        qpk = psqk.tile([128, 512], F32, tag="qkp",
                                        name="qpk")
                        for dc in range(NDC):
                            nc.tensor.matmul(
                                qpq[:], wq_sb[:, dc, 0:128],
                                xt[:, dc, ts_],
                                start=(dc == 0), stop=(dc == NDC - 1))
                            nc.tensor.matmul(
                                qpk[:], wk_sb[:, dc, 0:128],
                                xt[:, dc, ts_],
                                start=(dc == 0), stop=(dc == NDC - 1))
                        nc.vector.tensor_copy(qt[0][:, ts_], qpq[:])
                        nc.vector.tensor_copy(kt[0][:, ts_], qpk[:])

                    def v_proj_a(ti):
                        vpt = psv.tile([128, 256], F32, tag="vp", name="vpt")
                        for dc in range(NDC):
                            nc.tensor.matmul(
                                vpt[:], xt[:, dc, ti * 128:(ti + 1) * 128],
                                wv_sb[:, dc, :],
                                start=(dc == 0), stop=(dc == NDC - 1))
                        nc.vector.tensor_copy(
                            vones[:, ti, :, 0:DH],
                            vpt[:].rearrange("p (h d) -> p h d", h=HPC))

                    tb_chains(0)
                    v_proj_a(0)
                    # early attention blocks (0,0) and (0,1) run in a small
                    # single-buffered PSUM scope alongside the remaining
                    # projection chains, which are hooked into their chunk
                    # slots as PE fillers.
                    with tc.tile_pool(name="psB0", bufs=1,
                                      space="PSUM") as psb0, \
                         tc.tile_pool(name="psY0", bufs=1,
                                      space="PSUM") as psy0:
                        def stab0():
                            return psb0.tile([128, 2, 512], F32,
                                             tag="stab0", name="st0")

                        yab00 = psy0.tile([128, 2, 512], F32, tag="yab0")
                        emit_block(0, 0, yab00, stab0,
                                   lambda jc: v_proj_a(jc + 1)
                                   if jc <= 2 else None)
                        tb_chains(1)
                        # renorm(0,0) here: its ACT chain overlaps tb1
                        make_renorm(0, 0, yab00)(psqk, "qkp")

                        yab01 = psy0.tile([128, 2, 512], F32, tag="yab0",
                                          name="yab01")
                        hooks01 = {0: lambda: (v_proj_a(4), v_proj_a(5)),
                                   1: lambda: (v_proj_a(6), v_proj_a(7)),
                                   2: lambda: tb_chains(2),
                                   4: lambda: tb_chains(3)}
                        emit_block(0, 1, yab01, stab0,
                                   lambda jc: hooks01.pop(jc, lambda: 0)())
                        # must complete inside this scope (its tiles die
                        # with the pool); the Ln/exp chain overlaps the
                        # hooked tb3/v chains still draining on the PE
                        make_renorm(0, 1, yab01)(psb0, "stab0")

            # ------------- phase B main: blocks (0,1)..(1,3) -------
                with tc.tile_pool(name="psBst", bufs=2, space="PSUM") as psbst, \
                     tc.tile_pool(name="psBy", bufs=2, space="PSUM") as psby:
                    def v_proj_b(ti):
                        vp = psbst.tile([128, 256], F32, tag="stab",
                                        name="vpb")
                        for dc in range(NDC):
                            nc.tensor.matmul(
                                vp[:], xt[:, dc, ti * 128:(ti + 1) * 128],
                                wv_sb[:, dc, :],
                                start=(dc == 0), stop=(dc == NDC - 1))
                        nc.vector.tensor_copy(
                            vones[:, ti, :, 0:DH],
                            vp[:].rearrange("p (h d) -> p h d", h=HPC))

                    def qk_chain(w_sb, dst, pi, ib):
                        qp = psby.tile([128, 512], F32, tag="yab",
                                       name="qpc")
                        for dc in range(NDC):
                            nc.tensor.matmul(
                                qp[:], w_sb[:, dc, pi * 128:(pi + 1) * 128],
                                xt[:, dc, ib * 512:(ib + 1) * 512],
                                start=(dc == 0), stop=(dc == NDC - 1))
                        nc.vector.tensor_copy(
                            dst[pi][:, ib * 512:(ib + 1) * 512], qp[:])

                    ostg = {}

                    def c_pair(g, tq):
                        # out-projection for t-tile 4g+tq (both halves)
                        if tq == 0:
                            ostg[g] = pc_.tile([128, 4, D], BF16, tag="ostg",
                                               name=f"ostg{g}")
                        ti = 4 * g + tq
                        for eh in range(2):
                            op = psby.tile([128, 512], F32, tag="yab",
                                           name=f"op{eh}")
                            for pi in range(2):
                                nc.tensor.matmul(
                                    op[:],
                                    ypair[pi][:, ti * 128:(ti + 1) * 128],
                                    wo_sb[:, pi, eh * 512:(eh + 1) * 512],
                                    start=(pi == 0), stop=(pi == 1))
                            nc.vector.tensor_copy(
                                ostg[g][:, tq, eh * 512:(eh + 1) * 512],
                                op[:])
                        if g == NIB - 1:  # tail group: store per-ti
                            nc.sync.dma_start(
                                OUT[:, ti:ti + 1, :],
                                ostg[g][:, tq:tq + 1, :])
                        elif tq == 3:
                            nc.sync.dma_start(
                                OUT[:, 4 * g:4 * g + 4, :], ostg[g][:])

                    # v 8-11 must precede block (0,2)'s later PV chunks
                    for t in (8, 9, 10, 11):
                        v_proj_b(t)
                    FILLERS = {
                        (0, 2): [lambda: [v_proj_b(t) for t in (12, 13)],
                                 lambda: [v_proj_b(t) for t in (14, 15)],
                                 lambda: qk_chain(wq_sb, qt, 1, 0),
                                 lambda: qk_chain(wk_sb, kt, 1, 0)],
                        (0, 3): [lambda: qk_chain(wq_sb, qt, 1, 1),
                                 lambda: qk_chain(wk_sb, kt, 1, 1)],
                        (1, 0): [lambda: qk_chain(wq_sb, qt, 1, 2),
                                 lambda: qk_chain(wk_sb, kt, 1, 2)],
                        (1, 1): [lambda: qk_chain(wq_sb, qt, 1, 3),
                                 lambda: qk_chain(wk_sb, kt, 1, 3)],
                    }
                    state = {"pending": None, "cgrp": None,
                             "cslots": ()}

                    def hook(pi, ib):
                        def h(jc):
                            if jc == 1 and state["pending"] is not None:
                                state["pending"](psbst, "stab")
                                state["pending"] = None
                                if pi == 1 and ib >= 1:
                                    state["cgrp"] = ib - 1
                                    state["cslots"] = (
                                        (3, 5, 6, 7) if JPB * ib + JPB < 10
                                        else (3, 5, 7, 9))
                            if (state["cgrp"] is not None
                                    and jc in state["cslots"]):
                                c_pair(state["cgrp"],
                                       state["cslots"].index(jc))
                        return h

                    for pi in range(2):
                        for ib in range(NIB):
                            if pi == 0 and ib <= 1:
                                continue  # emitted early in phase A
                            yab = psby.tile([128, 2, 512], F32, tag="yab",
                                            name="yab")
                            emit_block(
                                pi, ib, yab,
                                lambda: psbst.tile([128, 2, 512], F32,
                                                   tag="stab", name="stab"),
                                hook(pi, ib))
                            state["pending"] = make_renorm(pi, ib, yab)
                            for fn in FILLERS.get((pi, ib), ()):
                                fn()
                    state["pending"](psbst, "stab")
                    for tq in range(4):
                        c_pair(NIB - 1, tq)

    nc.compile()
    return nc


_NC = None


def build_in_maps(x, w_qkv, w_out):
    x = np.asarray(x, np.float32)
    w_qkv = np.asarray(w_qkv, np.float32)
    w_out = np.asarray(w_out, np.float32)

    tri = np.triu(np.ones((128, 128), np.float32)).astype(
        ml_dtypes.bfloat16)                                # tri[j,i]=1 iff j<=i

    # [d, n] -> [128, d//128, n] with partition p: d = chunk*128 + p
    def dswz(w, dt=np.float32):
        return np.ascontiguousarray(
            w.reshape(NDC, 128, -1).transpose(1, 0, 2)).astype(dt)

    in_maps = []
    for c in range(NCORES):
        b, g = divmod(c, 4)
        cs = slice(g * 256, (g + 1) * 256)
        in_maps.append({
            "XT": dswz(np.ascontiguousarray(x[b].T), ml_dtypes.bfloat16),
            "WQ": dswz(np.ascontiguousarray(w_qkv[:, 0:1024][:, cs]),
                       ml_dtypes.bfloat16),
            "WK": dswz(np.ascontiguousarray(w_qkv[:, 1024:2048][:, cs]),
                       ml_dtypes.bfloat16),
            "WV": dswz(np.ascontiguousarray(w_qkv[:, 2048:3072][:, cs]),
                       ml_dtypes.bfloat16),
            "WO": np.ascontiguousarray(
                w_out[g * 256:(g + 1) * 256, :].reshape(2, 128, D)
                .transpose(1, 0, 2)).astype(ml_dtypes.bfloat16),
            "TRI": tri,
        })
    return in_maps


def kernel(x, w_qkv, w_out):
    global _NC
    if _NC is None:
        _NC = _build()

    in_maps = build_in_maps(x, w_qkv, w_out)
    res = run_bass_kernel_spmd(_NC, in_maps, core_ids=list(range(NCORES)))
    # OUT is [128, NTT, D] with row t = ti*128 + p -> unswizzle to [T, D]
    outs = [res.results[c]["OUT"].astype(np.float32)
            .transpose(1, 0, 2).reshape(T, D) for c in range(NCORES)]
    y = np.stack([outs[0] + outs[1] + outs[2] + outs[3],
                  outs[4] + outs[5] + outs[6] + outs[7]], axis=0)
    return y.astype(np.float32)
